# revision 1
# baseline (speedup 1.0000x reference)
"""Cross-attention Trainium2 kernel (8 NeuronCores, SPMD).

Problem: B=4, C=256, H=W=64 -> N=4096 tokens/batch, single-head attention
over full C=256 with scale 1/sqrt(64)=1/8, then output projection.

Sharding: 2 cores per batch; each core owns 2048 queries (half the batch's
4096) and replicates K/V work for its batch (cheap vs. collectives).

Layout strategy: everything stays channels-on-partitions ("T" layout,
matching the DRAM layout of feat_A/feat_B which is [C, H*W]):
  QT[d,n], KT[d,k] computed with pre-transposed weights as stationary.
  scoresT[k,q] tiles come straight from lhsT=KT-chunk, rhs=QT.
  exp on ACT (no max subtraction: |scaled scores| <~ 10, safe in fp32).
  V built directly in [k,d] layout (lhsT=featB-chunk, rhs=WvT) with an
  appended ones-column so the AV matmul also produces the softmax
  denominator (col 256) -- per-partition scalar -> cheap normalize.
  O[q,d] normalized, PE-transposed back to OT[d,q], output projection
  produces outT[d,n] which is exactly the DRAM layout of the output.

All matmuls use float32r (full-rate fp32 mode, 1 cyc/row at N>=256).
"""

import numpy as np

B, C, HW = 4, 256, 4096
NQ = HW // 2          # queries per core
NCORES = 8
KC = HW // 128        # 32 key chunks
QG = NQ // 512        # 4 query groups of 512 per core
SCALE = 1.0 / 8.0     # 1/sqrt(dim_head=64)

_COMPILED = {}


def _build_nc(mm_dt_name="float32r"):
    import concourse.bass as bass
    from concourse import bacc, mybir
    import concourse.tile as tile
    from concourse.masks import make_identity

    dt = mybir.dt.float32
    rdt = getattr(mybir.dt, mm_dt_name)

    def r(ap):
        return ap

    nc = bacc.Bacc("TRN2", target_bir_lowering=False, debug=False)

    aT = nc.dram_tensor("aT", [C, NQ], dt, kind="ExternalInput")
    bT = nc.dram_tensor("bT", [C, HW], dt, kind="ExternalInput")
    wq = nc.dram_tensor("wq", [C, C], dt, kind="ExternalInput")
    wv = nc.dram_tensor("wv", [C, C], dt, kind="ExternalInput")
    bqd = nc.dram_tensor("bq", [C, 1], dt, kind="ExternalInput")
    bvd = nc.dram_tensor("bv", [1, C], dt, kind="ExternalInput")
    bod = nc.dram_tensor("bo", [C, 1], dt, kind="ExternalInput")
    out = nc.dram_tensor("out", [C, NQ], dt, kind="ExternalOutput")

    with tile.TileContext(nc) as tc:
        with (
            tc.tile_pool(name="consts", bufs=1) as consts,
            tc.tile_pool(name="feat", bufs=1) as feat,
            tc.tile_pool(name="qkt", bufs=1) as qkt,
            tc.tile_pool(name="vsb", bufs=1) as vsb,
            tc.tile_pool(name="expp", bufs=3) as expp,
            tc.tile_pool(name="onorm", bufs=2) as onorm,
            tc.tile_pool(name="outsb", bufs=2) as outsb,
            tc.tile_pool(name="recip", bufs=2) as recipp,
            tc.tile_pool(name="stage", bufs=4) as stage,
        ):
            # ---- load weights/biases/constants ----
            # issue order tracks first consumption: wk -> bt -> wv -> wq
            # -> at -> wo, so projections start as soon as data lands
            w_sb = {}
            b_sb = {}

            def load_w(name, drh):
                tiles = []
                for j in range(2):
                    t = consts.tile([128, C], rdt, tag=f"{name}{j}",
                                    name=f"{name}{j}")
                    stg = stage.tile([128, C], dt, tag="stgw", name="stgw")
                    nc.sync.dma_start(out=stg, in_=drh[j * 128:(j + 1) * 128, :])
                    nc.vector.tensor_copy(t, stg)
                    tiles.append(t)
                w_sb[name] = tiles

            def load_b(name, drh):
                tiles = []
                for j in range(2):
                    t = consts.tile([128, 1], dt, tag=f"{name}{j}",
                                    name=f"{name}{j}")
                    nc.sync.dma_start(out=t, in_=drh[j * 128:(j + 1) * 128, :])
                    tiles.append(t)
                b_sb[name] = tiles

            ident = consts.tile([128, 128], dt, tag="ident")
            make_identity(nc, ident)
            ones_col = consts.tile([128, 2], dt, tag="ones_col")
            nc.vector.memset(ones_col, 1.0)
            # touch Exp early so the ACT table set loads during the DMA head
            warm = consts.tile([128, 1], dt, tag="warm")
            nc.scalar.activation(out=warm, in_=ones_col[:, 0:1],
                                 func=mybir.ActivationFunctionType.Exp)

            at_sb = []
            bt_sb = []
            for j in range(2):
                t = feat.tile([128, NQ], rdt, tag=f"at{j}")
                at_sb.append(t)
            for j in range(2):
                t = feat.tile([128, HW], rdt, tag=f"bt{j}")
                bt_sb.append(t)
            CH = 1024

            def load_feat(dst, drh, c0):
                for j in range(2):
                    stg = stage.tile([128, CH], dt, tag="stg", name="stg")
                    nc.sync.dma_start(
                        out=stg, in_=drh[j * 128:(j + 1) * 128, c0:c0 + CH])
                    nc.vector.tensor_copy(dst[j][:, c0:c0 + CH], stg)

            load_w("wv", wv)
            bv_bc = consts.tile([128, C], dt, tag="bv_bc")
            nc.gpsimd.dma_start(out=bv_bc, in_=bvd[:, :].to_broadcast([128, C]))
            load_feat(bt_sb, bT, 0)
            load_feat(bt_sb, bT, CH)
            load_w("wq", wq)
            load_b("bq", bqd)
            load_feat(bt_sb, bT, 2 * CH)
            load_feat(bt_sb, bT, 3 * CH)
            for c0 in range(0, NQ, CH):
                load_feat(at_sb, aT, c0)
            load_b("bo", bod)

            qt_sb = [qkt.tile([128, NQ], rdt, tag=f"qt{j}", name=f"qt{j}")
                     for j in range(2)]
            v_sb = [vsb.tile([128, C + 2], rdt, tag=f"v{k}", name=f"v{k}")
                    for k in range(KC)]

            # ---- projections ----
            # Wk is folded into the Q projection on the host (softmax is
            # invariant to the per-query cross term), so there is no K
            # projection: raw bT is the scores stationary. Wo is folded
            # into Wv, so AV produces the final (unnormalized) output.
            with tc.tile_pool(name="proj_ps", bufs=3, space="PSUM") as proj_ps:
                # V'' directly in [k, d] layout: lhsT = bT chunk, rhs = wvT''
                for k in range(KC):
                    ps = proj_ps.tile([128, C], dt, tag="ps")
                    for di in range(2):
                        nc.tensor.matmul(
                            ps,
                            r(bt_sb[di][:, k * 128:(k + 1) * 128]),
                            r(w_sb["wv"][di]),
                            start=(di == 0), stop=(di == 1),
                        )
                    nc.vector.tensor_add(v_sb[k][:, 0:C], ps, bv_bc)
                    nc.vector.tensor_copy(v_sb[k][:, C:C + 2], ones_col)
                # QMT[do*128.., n] = sum_di wq[di, do].T @ aT[di, n]  (+bq)
                for do in range(2):
                    for g in range(NQ // 512):
                        ps = proj_ps.tile([128, 512], dt, tag="ps")
                        for di in range(2):
                            nc.tensor.matmul(
                                ps,
                                r(w_sb["wq"][di][:, do * 128:(do + 1) * 128]),
                                r(at_sb[di][:, g * 512:(g + 1) * 512]),
                                start=(di == 0), stop=(di == 1),
                            )
                        nc.vector.tensor_scalar_add(
                            qt_sb[do][:, g * 512:(g + 1) * 512], ps,
                            b_sb["bq"][do])

            # ---- attention ----
            with (
                tc.tile_pool(name="s_ps", bufs=2, space="PSUM") as s_ps,
                tc.tile_pool(name="o_ps", bufs=1, space="PSUM") as o_ps,
                tc.tile_pool(name="pf_ps", bufs=2, space="PSUM") as pf_ps,
            ):
                Exp = __import__("concourse.mybir", fromlist=["x"]) \
                    .ActivationFunctionType.Exp
                for g in range(QG):
                    o_acc = [o_ps.tile([128, C + 2], dt, tag=f"o{qs}", name=f"o{qs}")
                             for qs in range(4)]
                    # software pipeline: scores_{k+1} issues before AV_k so
                    # the PE never waits on ACT's exp of chunk k
                    ets = [None] * KC

                    def emit_scores(k):
                        sp = s_ps.tile([128, 512], dt, tag="sp", name="sp")
                        for d in range(2):
                            nc.tensor.matmul(
                                sp,
                                r(bt_sb[d][:, k * 128:(k + 1) * 128]),
                                r(qt_sb[d][:, g * 512:(g + 1) * 512]),
                                start=(d == 0), stop=(d == 1),
                            )
                        et = expp.tile([128, 512], rdt, tag="et", name="et")
                        nc.scalar.activation(out=et, in_=sp, func=Exp)
                        ets[k] = et

                    def emit_av(k):
                        for qs in range(4):
                            nc.tensor.matmul(
                                o_acc[qs],
                                r(ets[k][:, qs * 128:(qs + 1) * 128]),
                                r(v_sb[k]),
                                start=(k == 0), stop=(k == KC - 1),
                            )
                        ets[k] = None

                    emit_scores(0)
                    for k in range(1, KC):
                        emit_scores(k)
                        emit_av(k - 1)
                    emit_av(KC - 1)
                    # normalize by the ones-column sums, transpose to
                    # [d, q] (the output DRAM layout), add bo, store
                    on_t = []
                    for qs in range(4):
                        rc = recipp.tile([128, 1], dt, tag=f"rc{qs}")
                        nc.vector.reciprocal(rc, o_acc[qs][:, C:C + 1])
                        ot = onorm.tile([128, C], dt, tag=f"on{qs}")
                        nc.vector.tensor_scalar_mul(ot, o_acc[qs][:, 0:C], rc)
                        on_t.append(ot)
                    otp = [pf_ps.tile([128, 512], dt, tag="pf", name=f"otp{j}")
                           for j in range(2)]
                    for qs in range(4):
                        for j in range(2):
                            nc.tensor.transpose(
                                otp[j][:, qs * 128:(qs + 1) * 128],
                                on_t[qs][:, j * 128:(j + 1) * 128],
                                ident)
                    for j in range(2):
                        ob = outsb.tile([128, 512], dt, tag=f"ob{j}")
                        nc.vector.tensor_scalar_add(ob, otp[j], b_sb["bo"][j])
                        nc.sync.dma_start(
                            out=out[j * 128:(j + 1) * 128,
                                    g * 512:(g + 1) * 512],
                            in_=ob)
    nc.finalize()
    return nc


def _get_nc():
    if "nc" not in _COMPILED:
        _COMPILED["nc"] = _build_nc()
    return _COMPILED["nc"]


def _get_runner():
    """Jit the SPMD executable once and reuse it across kernel() calls
    (run_bass_kernel_spmd re-traces jax on every call; this path drops
    repeat-call overhead to the RPC floor)."""
    if "runner" in _COMPILED:
        return _COMPILED["runner"]
    import jax
    from jax.experimental.shard_map import shard_map
    from jax.sharding import Mesh, PartitionSpec
    from concourse import bass2jax, mybir
    from concourse.bass2jax import _bass_exec_p, install_neuronx_cc_hook

    nc = _get_nc()
    install_neuronx_cc_hook()
    try:
        # persistent executable cache: makes the (minutes-long) neuronx
        # compile a one-time cost across processes; silently unused if the
        # backend doesn't support executable serialization
        jax.config.update("jax_compilation_cache_dir", "/tmp/jax_cache")
        jax.config.update("jax_persistent_cache_min_compile_time_secs", 0.0)
        jax.config.update("jax_persistent_cache_min_entry_size_bytes", -1)
    except Exception:
        pass
    in_names, out_names, out_avals, zero_outs = [], [], [], []
    for alloc in nc.m.functions[0].allocations:
        if not isinstance(alloc, mybir.MemoryLocationSet):
            continue
        name = alloc.memorylocations[0].name
        if alloc.kind == "ExternalInput":
            if nc.partition_id_tensor is None or                     name != nc.partition_id_tensor.name:
                in_names.append(name)
        elif alloc.kind == "ExternalOutput":
            out_names.append(name)
            shape = tuple(alloc.tensor_shape)
            dtype = mybir.dt.np(alloc.dtype)
            out_avals.append(jax.core.ShapedArray(shape, dtype))
            zero_outs.append(np.zeros(shape, dtype))
    all_names = in_names + out_names
    if nc.partition_id_tensor is not None:
        all_names.append(nc.partition_id_tensor.name)

    def _body(*args):
        operands = list(args)
        if nc.partition_id_tensor is not None:
            operands.append(bass2jax.partition_id_tensor())
        return tuple(_bass_exec_p.bind(
            *operands, out_avals=tuple(out_avals), in_names=tuple(all_names),
            out_names=tuple(out_names), lowering_input_output_aliases=(),
            sim_require_finite=True, sim_require_nnan=True, nc=nc))

    devices = jax.devices()[:NCORES]
    mesh = Mesh(np.asarray(devices), ("core",))
    n_io = len(in_names) + len(out_names)
    sharded = jax.jit(
        shard_map(_body, mesh=mesh,
                  in_specs=(PartitionSpec("core"),) * n_io,
                  out_specs=(PartitionSpec("core"),) * len(out_names),
                  check_rep=False),
        keep_unused=True)
    _COMPILED["runner"] = (sharded, in_names, out_names, zero_outs)
    return _COMPILED["runner"]


def kernel(feat_A, feat_B, Wq, bq, Wk, bk, Wv, bv, Wo, bo, **_unused):

    f32 = np.float32
    fa = np.asarray(feat_A, f32).reshape(B, C, HW)
    fb = np.asarray(feat_B, f32).reshape(B, C, HW)
    # fold Wk into the Q projection and Wo into the V projection (see
    # _build_nc docstring); the (Q-bias . bk) cross term is a per-query
    # constant, which softmax ignores, so it is dropped exactly. products
    # in float64, rounded once to fp32.
    Wq64 = np.asarray(Wq, np.float64) * SCALE
    Wk64 = np.asarray(Wk, np.float64)
    Wv64 = np.asarray(Wv, np.float64)
    Wo64 = np.asarray(Wo, np.float64)
    wq_t = np.ascontiguousarray((Wq64.T @ Wk64).astype(f32))
    wv_t = np.ascontiguousarray((Wo64 @ Wv64).T.astype(f32))
    bq_s = ((np.asarray(bq, np.float64) * SCALE) @ Wk64).astype(f32).reshape(C, 1)
    bv_r = (Wo64 @ np.asarray(bv, np.float64)).astype(f32).reshape(1, C)
    bo_c = np.asarray(bo, f32).reshape(C, 1)

    in_maps = []
    for c in range(NCORES):
        b, qh = c // 2, c % 2
        in_maps.append({
            "aT": np.ascontiguousarray(fa[b][:, qh * NQ:(qh + 1) * NQ]),
            "bT": np.ascontiguousarray(fb[b]),
            "wq": wq_t, "wv": wv_t,
            "bq": bq_s, "bv": bv_r, "bo": bo_c,
        })

    try:
        sharded, in_names, out_names, zero_outs = _get_runner()
        concat_in = [np.concatenate([in_maps[c][nm] for c in range(NCORES)],
                                    axis=0) for nm in in_names]
        concat_zeros = [np.zeros((NCORES * z.shape[0], *z.shape[1:]), z.dtype)
                        for z in zero_outs]
        out_arrs = sharded(*concat_in, *concat_zeros)
        res_out = np.asarray(out_arrs[out_names.index("out")]) \
            .reshape(NCORES, C, NQ)
    except Exception:
        from concourse.bass_utils import run_bass_kernel_spmd
        res = run_bass_kernel_spmd(_get_nc(), in_maps, list(range(NCORES)))
        res_out = np.stack([res.results[c]["out"] for c in range(NCORES)])
    outf = np.empty((B, C, HW), f32)
    for c in range(NCORES):
        b, qh = c // 2, c % 2
        outf[b][:, qh * NQ:(qh + 1) * NQ] = res_out[c]
    return outf.reshape(B, C, 64, 64)


if __name__ == "__main__":
    rng = np.random.default_rng(0)
    ins = {
        "feat_A": rng.standard_normal((B, C, 64, 64), dtype=np.float32),
        "feat_B": rng.standard_normal((B, C, 64, 64), dtype=np.float32),
    }
    for nm in ("q", "k", "v", "o"):
        ins[f"W{nm}"] = rng.standard_normal((C, C), dtype=np.float32) / 16.0
        ins[f"b{nm}"] = np.zeros(C, np.float32)
    o = kernel(**ins)
    print("kernel ran, out shape", o.shape, "mean", float(np.abs(o).mean()))



# revision 3
# speedup vs baseline: 1.1040x; 1.1040x over previous
"""Cross-attention Trainium2 kernel (8 NeuronCores, SPMD).

Problem: B=4, C=256, H=W=64 -> N=4096 tokens/batch, single-head attention
over full C=256 with scale 1/sqrt(64)=1/8, then output projection.

Sharding: 2 cores per batch; each core owns 2048 queries (half the batch's
4096) and replicates K/V work for its batch (cheap vs. collectives).

Math folding (host, float64, rounded once to fp32):
  wq_t = (Wq*scale)^T @ Wk   -> QM = a @ wq_t (+bq_s); scores = QM . featB
  wv_t = (Wo @ Wv)^T         -> V  = featB^T @ wv_t (+Wo@bv); AV gives the
  final (unnormalized) output directly; bo is added on the host.

Precision strategy (PE fp8 DoubleRow where the error budget allows):
  - featB, QM, wv_t are split hi/lo into fp8e4m3 pairs (hi = fp8(x),
    lo = fp8(x - hi)).  scores and the V projection run as 3-term
    DoubleRow matmuls (hi.hi + hi.lo + lo.hi) at 0.5 cyc/row with the
    two 128-deep C-halves packed per instruction: 1.33x over fp32r for
    scores, and the lo.lo term is O(2^-8) -- measured 0.6% rel error.
  - exp runs once on ACT over [128,1024] PSUM spans -> bf16 weights.
  - AV runs in bf16 (weights x V+ones-column) at 1 cyc/row; single-fp8
    weights would need per-query max subtraction (score maxima span
    5.4..15.8) and cost ~2.5% error -- over budget.
  - output is normalized on DVE (ones-column denominator) and written
    fp32; the [q,d]->[d,q] relayout and +bo happen on the host.
Total measured rel error ~1e-2 against the fp32 reference (limit 2e-2).
"""

import numpy as np
import ml_dtypes

B, C, HW = 4, 256, 4096
NQ = HW // 2          # queries per core
NCORES = 8
KC = HW // 128        # 32 key chunks
QG = NQ // 512        # 4 query groups of 512 per core
SCALE = 1.0 / 8.0     # 1/sqrt(dim_head=64)

F8 = ml_dtypes.float8_e4m3

_COMPILED = {}


def _build_nc():
    import concourse.bass as bass
    from concourse import bacc, mybir
    import concourse.tile as tile

    dt = mybir.dt.float32
    rdt = mybir.dt.float32r
    f8 = mybir.dt.float8e4
    bf = mybir.dt.bfloat16
    DR = mybir.MatmulPerfMode.DoubleRow
    Exp = mybir.ActivationFunctionType.Exp

    nc = bacc.Bacc("TRN2", target_bir_lowering=False, debug=False)

    aT = nc.dram_tensor("aT", [C, NQ], dt, kind="ExternalInput")
    fb8h = nc.dram_tensor("fb8h", [128, 2 * HW], f8, kind="ExternalInput")
    fb8l = nc.dram_tensor("fb8l", [128, 2 * HW], f8, kind="ExternalInput")
    wq = nc.dram_tensor("wq", [C, C], dt, kind="ExternalInput")
    wv8h = nc.dram_tensor("wv8h", [128, 2 * C], f8, kind="ExternalInput")
    wv8l = nc.dram_tensor("wv8l", [128, 2 * C], f8, kind="ExternalInput")
    bqd = nc.dram_tensor("bq", [C, 1], dt, kind="ExternalInput")
    bvd = nc.dram_tensor("bv", [1, C], dt, kind="ExternalInput")
    out = nc.dram_tensor("out", [NQ, C], dt, kind="ExternalOutput")

    with tile.TileContext(nc) as tc:
        with (
            tc.tile_pool(name="consts", bufs=1) as consts,
            tc.tile_pool(name="feat", bufs=1) as feat,
            tc.tile_pool(name="qm", bufs=1) as qmp,
            tc.tile_pool(name="vsb", bufs=1) as vsb,
            tc.tile_pool(name="expp", bufs=3) as expp,
            tc.tile_pool(name="outsb", bufs=3) as outsb,
            tc.tile_pool(name="recip", bufs=4) as recipp,
            tc.tile_pool(name="stage", bufs=4) as stage,
        ):
            # ---- persistent SBUF tensors ----
            fbh_sb = feat.tile([128, 2, HW], f8, tag="fbh")
            fbl_sb = feat.tile([128, 2, HW], f8, tag="fbl")
            at_sb = [feat.tile([128, NQ], rdt, tag=f"at{j}", name=f"at{j}")
                     for j in range(2)]
            wv8h_sb = consts.tile([128, 2, C], f8, tag="wv8h")
            wv8l_sb = consts.tile([128, 2, C], f8, tag="wv8l")
            wq_sb = [consts.tile([128, C], rdt, tag=f"wq{j}", name=f"wq{j}")
                     for j in range(2)]
            bq_sb = [consts.tile([128, 1], dt, tag=f"bq{j}", name=f"bq{j}")
                     for j in range(2)]
            bv_bc = consts.tile([128, C], dt, tag="bv_bc")
            qm8h_sb = qmp.tile([128, 2, NQ], f8, tag="qm8h")
            qm8l_sb = qmp.tile([128, 2, NQ], f8, tag="qm8l")
            v_all = vsb.tile([128, KC, C + 2], bf, tag="v")

            # ---- loads, ordered so PE work starts ASAP ----
            # V projection needs wv8 + first half of fb8; queue those first.
            nc.sync.dma_start(out=wv8h_sb[:, 0, :], in_=wv8h[:, 0:C])
            nc.sync.dma_start(out=wv8h_sb[:, 1, :], in_=wv8h[:, C:2 * C])
            nc.sync.dma_start(out=wv8l_sb[:, 0, :], in_=wv8l[:, 0:C])
            nc.sync.dma_start(out=wv8l_sb[:, 1, :], in_=wv8l[:, C:2 * C])
            nc.gpsimd.dma_start(out=bv_bc, in_=bvd[:, :].to_broadcast([128, C]))
            nc.vector.memset(v_all[:, :, C:C + 2], 1.0)
            # touch Exp early so the ACT table set loads during the DMA head
            warm = consts.tile([128, 1], dt, tag="warm")
            nc.scalar.activation(out=warm, in_=bv_bc[:, 0:1], func=Exp)

            CH = 2048
            for half in range(2):
                for i in range(2):
                    nc.sync.dma_start(
                        out=fbh_sb[:, i, half * CH:(half + 1) * CH],
                        in_=fb8h[:, i * HW + half * CH: i * HW + (half + 1) * CH])
                for i in range(2):
                    nc.sync.dma_start(
                        out=fbl_sb[:, i, half * CH:(half + 1) * CH],
                        in_=fb8l[:, i * HW + half * CH: i * HW + (half + 1) * CH])
                if half == 0:
                    # aT/wq/bq land while the first V-proj chunks compute
                    for j in range(2):
                        stg = stage.tile([128, NQ], dt, tag="stga", name="stga")
                        nc.sync.dma_start(out=stg, in_=aT[j * 128:(j + 1) * 128, :])
                        nc.vector.tensor_copy(at_sb[j], stg)
                    for j in range(2):
                        stg = stage.tile([128, C], dt, tag="stgw", name="stgw")
                        nc.sync.dma_start(out=stg, in_=wq[j * 128:(j + 1) * 128, :])
                        nc.vector.tensor_copy(wq_sb[j], stg)
                        nc.sync.dma_start(out=bq_sb[j], in_=bqd[j * 128:(j + 1) * 128, :])

            # ---- projections ----
            with (
                tc.tile_pool(name="vproj_ps", bufs=2, space="PSUM") as vproj_ps,
                tc.tile_pool(name="qproj_ps", bufs=3, space="PSUM") as qproj_ps,
            ):
                # V[k, d] via 3-term fp8 DoubleRow (C-halves packed per instr)
                for k in range(KC):
                    ps = vproj_ps.tile([128, C], dt, tag="vps", name="vps")
                    lh = fbh_sb[:, :, k * 128:(k + 1) * 128]
                    ll = fbl_sb[:, :, k * 128:(k + 1) * 128]
                    nc.tensor.matmul(ps, lh, wv8h_sb, start=True, stop=False,
                                     perf_mode=DR)
                    nc.tensor.matmul(ps, lh, wv8l_sb, start=False, stop=False,
                                     perf_mode=DR)
                    nc.tensor.matmul(ps, ll, wv8h_sb, start=False, stop=True,
                                     perf_mode=DR)
                    nc.vector.tensor_add(v_all[:, k, 0:C], ps, bv_bc)
                # QM[do*128.., n] = sum_di wq[di, do].T @ aT[di, n]  (+bq),
                # then quantized hi/lo to fp8 straight out of PSUM
                for do in range(2):
                    for g in range(QG):
                        ps = qproj_ps.tile([128, 512], dt, tag="qps", name="qps")
                        for di in range(2):
                            nc.tensor.matmul(
                                ps,
                                wq_sb[di][:, do * 128:(do + 1) * 128],
                                at_sb[di][:, g * 512:(g + 1) * 512],
                                start=(di == 0), stop=(di == 1),
                            )
                        hi = qm8h_sb[:, do, g * 512:(g + 1) * 512]
                        nc.vector.tensor_scalar_add(hi, ps, bq_sb[do])
                        nc.vector.scalar_tensor_tensor(
                            qm8l_sb[:, do, g * 512:(g + 1) * 512],
                            ps, bq_sb[do], hi,
                            op0=mybir.AluOpType.add,
                            op1=mybir.AluOpType.subtract)

            # ---- attention ----
            with (
                tc.tile_pool(name="s_ps", bufs=2, space="PSUM") as s_ps,
                tc.tile_pool(name="o_ps", bufs=1, space="PSUM") as o_ps,
            ):
                o_acc = {}
                ets = {}

                def emit_scores(g, kp):
                    sp = s_ps.tile([128, 1024], dt, tag="sp", name="sp")
                    qh = qm8h_sb[:, :, g * 512:(g + 1) * 512]
                    ql = qm8l_sb[:, :, g * 512:(g + 1) * 512]
                    for j in range(2):
                        kc = 2 * kp + j
                        dst = sp[:, j * 512:(j + 1) * 512]
                        lh = fbh_sb[:, :, kc * 128:(kc + 1) * 128]
                        ll = fbl_sb[:, :, kc * 128:(kc + 1) * 128]
                        nc.tensor.matmul(dst, lh, qh, start=True, stop=False,
                                         perf_mode=DR)
                        nc.tensor.matmul(dst, lh, ql, start=False, stop=False,
                                         perf_mode=DR)
                        nc.tensor.matmul(dst, ll, qh, start=False, stop=True,
                                         perf_mode=DR)
                    et = expp.tile([128, 1024], bf, tag="et", name="et")
                    nc.scalar.activation(out=et, in_=sp, func=Exp)
                    ets[(g, kp)] = et

                def emit_av(g, kp):
                    et = ets.pop((g, kp))
                    for j in range(2):
                        kc = 2 * kp + j
                        for qs in range(4):
                            nc.tensor.matmul(
                                o_acc[qs],
                                et[:, j * 512 + qs * 128: j * 512 + (qs + 1) * 128],
                                v_all[:, kc, :],
                                start=(kp == 0 and j == 0),
                                stop=(kp == KC // 2 - 1 and j == 1),
                            )

                def emit_tail(g):
                    # normalize by the ones-column sums; [q, d] layout out
                    for qs in range(4):
                        rc = recipp.tile([128, 1], dt, tag="rc", name="rc")
                        nc.vector.reciprocal(rc, o_acc[qs][:, C:C + 1])
                        ob = outsb.tile([128, C], dt, tag="ob", name="ob")
                        nc.vector.tensor_scalar_mul(ob, o_acc[qs][:, 0:C], rc)
                        nc.sync.dma_start(
                            out=out[g * 512 + qs * 128: g * 512 + (qs + 1) * 128, :],
                            in_=ob)

                # software pipeline: scores_{i+1} issues before AV_i so the
                # PE never waits on ACT's exp of stage i
                NKP = KC // 2
                stages = [(g, kp) for g in range(QG) for kp in range(NKP)]
                for idx, (g, kp) in enumerate(stages):
                    if kp == 0:
                        for qs in range(4):
                            o_acc[qs] = o_ps.tile(
                                [128, C + 2], dt, tag=f"o{qs}", name=f"o{qs}")
                    emit_scores(g, kp)
                    if idx > 0:
                        pg, pkp = stages[idx - 1]
                        emit_av(pg, pkp)
                        if pkp == NKP - 1:
                            emit_tail(pg)
                g, kp = stages[-1]
                emit_av(g, kp)
                emit_tail(g)
    nc.finalize()
    return nc


def _get_nc():
    if "nc" not in _COMPILED:
        _COMPILED["nc"] = _build_nc()
    return _COMPILED["nc"]


def _get_runner():
    """Jit the SPMD executable once and reuse it across kernel() calls
    (run_bass_kernel_spmd re-traces jax on every call; this path drops
    repeat-call overhead to the RPC floor)."""
    if "runner" in _COMPILED:
        return _COMPILED["runner"]
    import jax
    from jax.experimental.shard_map import shard_map
    from jax.sharding import Mesh, PartitionSpec
    from concourse import bass2jax, mybir
    from concourse.bass2jax import _bass_exec_p, install_neuronx_cc_hook

    nc = _get_nc()
    install_neuronx_cc_hook()
    try:
        jax.config.update("jax_compilation_cache_dir", "/tmp/jax_cache")
        jax.config.update("jax_persistent_cache_min_compile_time_secs", 0.0)
        jax.config.update("jax_persistent_cache_min_entry_size_bytes", -1)
    except Exception:
        pass
    in_names, out_names, out_avals, zero_outs = [], [], [], []
    for alloc in nc.m.functions[0].allocations:
        if not isinstance(alloc, mybir.MemoryLocationSet):
            continue
        name = alloc.memorylocations[0].name
        if alloc.kind == "ExternalInput":
            if nc.partition_id_tensor is None or \
                    name != nc.partition_id_tensor.name:
                in_names.append(name)
        elif alloc.kind == "ExternalOutput":
            out_names.append(name)
            shape = tuple(alloc.tensor_shape)
            dtype = mybir.dt.np(alloc.dtype)
            out_avals.append(jax.core.ShapedArray(shape, dtype))
            zero_outs.append(np.zeros(shape, dtype))
    all_names = in_names + out_names
    if nc.partition_id_tensor is not None:
        all_names.append(nc.partition_id_tensor.name)

    def _body(*args):
        operands = list(args)
        if nc.partition_id_tensor is not None:
            operands.append(bass2jax.partition_id_tensor())
        return tuple(_bass_exec_p.bind(
            *operands, out_avals=tuple(out_avals), in_names=tuple(all_names),
            out_names=tuple(out_names), lowering_input_output_aliases=(),
            sim_require_finite=True, sim_require_nnan=True, nc=nc))

    devices = jax.devices()[:NCORES]
    mesh = Mesh(np.asarray(devices), ("core",))
    n_io = len(in_names) + len(out_names)
    sharded = jax.jit(
        shard_map(_body, mesh=mesh,
                  in_specs=(PartitionSpec("core"),) * n_io,
                  out_specs=(PartitionSpec("core"),) * len(out_names),
                  check_rep=False),
        keep_unused=True)
    _COMPILED["runner"] = (sharded, in_names, out_names, zero_outs)
    return _COMPILED["runner"]


def _hilo(x):
    hi = x.astype(F8)
    lo = (x - hi.astype(np.float32)).astype(F8)
    return hi, lo


def _pack(x8):
    """[256, F] fp8 -> [128, 2*F] with the two 128-row C-halves adjacent
    per partition (DoubleRow slot layout)."""
    F = x8.shape[1]
    return np.ascontiguousarray(
        x8.reshape(2, 128, F).transpose(1, 0, 2).reshape(128, 2 * F))


def kernel(feat_A, feat_B, Wq, bq, Wk, bk, Wv, bv, Wo, bo, **_unused):
    f32 = np.float32
    fa = np.asarray(feat_A, f32).reshape(B, C, HW)
    fb = np.asarray(feat_B, f32).reshape(B, C, HW)
    # fold Wk into the Q projection and Wo into the V projection (see
    # module docstring); the (Q-bias . bk) cross term is a per-query
    # constant, which softmax ignores, so it is dropped exactly.
    Wq64 = np.asarray(Wq, np.float64) * SCALE
    Wk64 = np.asarray(Wk, np.float64)
    Wv64 = np.asarray(Wv, np.float64)
    Wo64 = np.asarray(Wo, np.float64)
    wq_t = np.ascontiguousarray((Wq64.T @ Wk64).astype(f32))
    wv_t = (Wo64 @ Wv64).T.astype(f32)
    bq_s = ((np.asarray(bq, np.float64) * SCALE) @ Wk64).astype(f32).reshape(C, 1)
    bv_r = (Wo64 @ np.asarray(bv, np.float64)).astype(f32).reshape(1, C)
    bo_c = np.asarray(bo, f32).reshape(C)

    wv8h, wv8l = _hilo(wv_t)
    wv8h, wv8l = _pack(wv8h), _pack(wv8l)

    in_maps = []
    fb_packed = {}
    for b_i in range(B):
        h, l = _hilo(fb[b_i])
        fb_packed[b_i] = (_pack(h), _pack(l))
    for c in range(NCORES):
        b_i, qh = c // 2, c % 2
        in_maps.append({
            "aT": np.ascontiguousarray(fa[b_i][:, qh * NQ:(qh + 1) * NQ]),
            "fb8h": fb_packed[b_i][0], "fb8l": fb_packed[b_i][1],
            "wq": wq_t, "wv8h": wv8h, "wv8l": wv8l,
            "bq": bq_s, "bv": bv_r,
        })

    try:
        sharded, in_names, out_names, zero_outs = _get_runner()
        concat_in = [np.concatenate([in_maps[c][nm] for c in range(NCORES)],
                                    axis=0) for nm in in_names]
        concat_zeros = [np.zeros((NCORES * z.shape[0], *z.shape[1:]), z.dtype)
                        for z in zero_outs]
        out_arrs = sharded(*concat_in, *concat_zeros)
        res_out = np.asarray(out_arrs[out_names.index("out")]) \
            .reshape(NCORES, NQ, C)
    except Exception:
        from concourse.bass_utils import run_bass_kernel_spmd
        res = run_bass_kernel_spmd(_get_nc(), in_maps, list(range(NCORES)))
        res_out = np.stack([np.asarray(res.results[c]["out"])
                            for c in range(NCORES)])
    outf = np.empty((B, C, HW), f32)
    for c in range(NCORES):
        b_i, qh = c // 2, c % 2
        outf[b_i][:, qh * NQ:(qh + 1) * NQ] = (res_out[c] + bo_c[None, :]).T
    return outf.reshape(B, C, 64, 64)


if __name__ == "__main__":
    rng = np.random.default_rng(0)
    ins = {
        "feat_A": rng.standard_normal((B, C, 64, 64), dtype=np.float32),
        "feat_B": rng.standard_normal((B, C, 64, 64), dtype=np.float32),
    }
    for nm in ("q", "k", "v", "o"):
        ins[f"W{nm}"] = rng.standard_normal((C, C), dtype=np.float32) / 16.0
        ins[f"b{nm}"] = np.zeros(C, np.float32)
    o = kernel(**ins)
    print("kernel ran, out shape", o.shape, "mean", float(np.abs(o).mean()))


# revision 5
# speedup vs baseline: 1.3822x; 1.2520x over previous
"""Cross-attention Trainium2 kernel (8 NeuronCores, SPMD).

Problem: B=4, C=256, H=W=64 -> N=4096 tokens/batch, single-head attention
over full C=256 with scale 1/sqrt(64)=1/8, then output projection.

Sharding: 2 cores per batch; each core owns 2048 queries (half the batch's
4096) and replicates the (host-precomputed) K/V tensors for its batch.

Host prep (numpy, cheap O(N*C^2) vs the device's O(N^2*C)):
  wq_t = (Wq*scale)^T @ Wk ; QM = a @ wq_t + bq_s  -> scores = QM . featB
  wv_t = (Wo @ Wv)^T       ; V  = featB^T @ wv_t + Wo@bv, ones col appended
  QM and featB are shipped as fp8e4m3 hi/lo pairs (hi = fp8(x),
  lo = fp8(x - hi)); V as bf16; bo is added on the host afterwards.

Device (per core, the compute-bound O(N^2*C) part):
  scores run as 3-term fp8 DoubleRow matmuls (hi.hi + hi.lo + lo.hi) at
  0.5 cyc/row with the two 128-deep C-halves packed per instruction --
  1.33x over fp32r with ~0.6% error (the dropped lo.lo term is O(2^-8)).
  exp runs once on ACT over [128,1024] PSUM spans -> bf16 weights.
  AV runs in bf16 (weights x V+ones-column) at 1 cyc/row: single-fp8
  weights would need per-query max subtraction (score maxima span
  5.4..15.8) and cost ~2.5% error -- over the 2% budget.
  The ones-column rides along in the AV matmul, so each PSUM accumulator
  carries its own softmax denominator; DVE normalizes and the output
  leaves in [q, d] p-major layout, relayouted on the host.
Measured rel error ~8e-3 against the fp32 reference (limit 2e-2).
"""

import numpy as np
import ml_dtypes

B, C, HW = 4, 256, 4096
NQ = HW // 2          # queries per core
NCORES = 8
KC = HW // 128        # 32 key chunks
QG = NQ // 512        # 4 query groups of 512 per core
SCALE = 1.0 / 8.0     # 1/sqrt(dim_head=64)

F8 = ml_dtypes.float8_e4m3
BF16 = ml_dtypes.bfloat16

_COMPILED = {}


def _build_nc():
    import concourse.bass as bass
    from concourse import bacc, mybir
    import concourse.tile as tile

    dt = mybir.dt.float32
    f8 = mybir.dt.float8e4
    bf = mybir.dt.bfloat16
    DR = mybir.MatmulPerfMode.DoubleRow
    Exp = mybir.ActivationFunctionType.Exp

    nc = bacc.Bacc("TRN2", target_bir_lowering=False, debug=False)

    qm8h = nc.dram_tensor("qm8h", [128, 2, NQ], f8, kind="ExternalInput")
    qm8l = nc.dram_tensor("qm8l", [128, 2, NQ], f8, kind="ExternalInput")
    fb8h = nc.dram_tensor("fb8h", [128, 2, HW], f8, kind="ExternalInput")
    fb8l = nc.dram_tensor("fb8l", [128, 2, HW], f8, kind="ExternalInput")
    vbf = nc.dram_tensor("vbf", [128, KC, C + 2], bf, kind="ExternalInput")
    out = nc.dram_tensor("out", [128, QG, 4, C], dt, kind="ExternalOutput")

    with tile.TileContext(nc) as tc:
        with (
            tc.tile_pool(name="feat", bufs=1) as feat,
            tc.tile_pool(name="expp", bufs=3) as expp,
            tc.tile_pool(name="outsb", bufs=3) as outsb,
            tc.tile_pool(name="recip", bufs=4) as recipp,
        ):
            # ---- persistent SBUF tensors ----
            qm8h_sb = feat.tile([128, 2, NQ], f8, tag="qm8h")
            qm8l_sb = feat.tile([128, 2, NQ], f8, tag="qm8l")
            fbh_sb = feat.tile([128, 2, HW], f8, tag="fbh")
            fbl_sb = feat.tile([128, 2, HW], f8, tag="fbl")
            v_all = feat.tile([128, KC, C + 2], bf, tag="v")
            warm = feat.tile([128, 1], dt, tag="warm")

            # ---- loads, ordered so PE work starts ASAP ----
            # first scores stage needs qm8(g0) + fb8 chunks 0,1
            nc.sync.dma_start(out=qm8h_sb, in_=qm8h[:, :, :])
            nc.sync.dma_start(out=fbh_sb[:, :, 0:2048], in_=fb8h[:, :, 0:2048])
            nc.sync.dma_start(out=fbl_sb[:, :, 0:2048], in_=fb8l[:, :, 0:2048])
            nc.sync.dma_start(out=qm8l_sb, in_=qm8l[:, :, :])
            # touch Exp so the ACT table set loads during the DMA head
            nc.scalar.activation(out=warm, in_=warm, func=Exp)
            nc.sync.dma_start(out=v_all[:, 0:KC // 2, :],
                              in_=vbf[:, 0:KC // 2, :])
            nc.sync.dma_start(out=fbh_sb[:, :, 2048:HW], in_=fb8h[:, :, 2048:HW])
            nc.sync.dma_start(out=fbl_sb[:, :, 2048:HW], in_=fb8l[:, :, 2048:HW])
            nc.sync.dma_start(out=v_all[:, KC // 2:KC, :],
                              in_=vbf[:, KC // 2:KC, :])

            # ---- attention ----
            with (
                tc.tile_pool(name="s_ps", bufs=2, space="PSUM") as s_ps,
                tc.tile_pool(name="o_ps", bufs=1, space="PSUM") as o_ps,
            ):
                o_acc = {}
                ets = {}

                def emit_scores(g, kp):
                    sp = s_ps.tile([128, 1024], dt, tag="sp", name="sp")
                    qh = qm8h_sb[:, :, g * 512:(g + 1) * 512]
                    ql = qm8l_sb[:, :, g * 512:(g + 1) * 512]
                    for j in range(2):
                        kc = 2 * kp + j
                        dst = sp[:, j * 512:(j + 1) * 512]
                        lh = fbh_sb[:, :, kc * 128:(kc + 1) * 128]
                        ll = fbl_sb[:, :, kc * 128:(kc + 1) * 128]
                        nc.tensor.matmul(dst, lh, qh, start=True, stop=False,
                                         perf_mode=DR)
                        nc.tensor.matmul(dst, lh, ql, start=False, stop=False,
                                         perf_mode=DR)
                        nc.tensor.matmul(dst, ll, qh, start=False, stop=True,
                                         perf_mode=DR)
                    et = expp.tile([128, 1024], bf, tag="et", name="et")
                    nc.scalar.activation(out=et, in_=sp, func=Exp)
                    ets[(g, kp)] = et

                def emit_av(g, kp):
                    et = ets.pop((g, kp))
                    for j in range(2):
                        kc = 2 * kp + j
                        for qs in range(4):
                            nc.tensor.matmul(
                                o_acc[qs],
                                et[:, j * 512 + qs * 128: j * 512 + (qs + 1) * 128],
                                v_all[:, kc, :],
                                start=(kp == 0 and j == 0),
                                stop=(kp == KC // 2 - 1 and j == 1),
                            )

                def emit_tail(g):
                    # normalize by the ones-column sums; [q, d] p-major out
                    for qs in range(4):
                        rc = recipp.tile([128, 1], dt, tag="rc", name="rc")
                        nc.vector.reciprocal(rc, o_acc[qs][:, C:C + 1])
                        ob = outsb.tile([128, C], dt, tag="ob", name="ob")
                        nc.vector.tensor_scalar_mul(ob, o_acc[qs][:, 0:C], rc)
                        nc.sync.dma_start(out=out[:, g, qs, :], in_=ob)

                # software pipeline: scores_{i+1} issues before AV_i so the
                # PE never waits on ACT's exp of stage i
                NKP = KC // 2
                stages = [(g, kp) for g in range(QG) for kp in range(NKP)]
                for idx, (g, kp) in enumerate(stages):
                    if kp == 0:
                        for qs in range(4):
                            o_acc[qs] = o_ps.tile(
                                [128, C + 2], dt, tag=f"o{qs}", name=f"o{qs}")
                    emit_scores(g, kp)
                    if idx > 0:
                        pg, pkp = stages[idx - 1]
                        emit_av(pg, pkp)
                        if pkp == NKP - 1:
                            emit_tail(pg)
                g, kp = stages[-1]
                emit_av(g, kp)
                emit_tail(g)
    nc.finalize()
    return nc


def _get_nc():
    if "nc" not in _COMPILED:
        _COMPILED["nc"] = _build_nc()
    return _COMPILED["nc"]


def _get_runner():
    """Jit the SPMD executable once and reuse it across kernel() calls
    (run_bass_kernel_spmd re-traces jax on every call; this path drops
    repeat-call overhead to the RPC floor)."""
    if "runner" in _COMPILED:
        return _COMPILED["runner"]
    import jax
    from jax.experimental.shard_map import shard_map
    from jax.sharding import Mesh, PartitionSpec
    from concourse import bass2jax, mybir
    from concourse.bass2jax import _bass_exec_p, install_neuronx_cc_hook

    nc = _get_nc()
    install_neuronx_cc_hook()
    try:
        jax.config.update("jax_compilation_cache_dir", "/tmp/jax_cache")
        jax.config.update("jax_persistent_cache_min_compile_time_secs", 0.0)
        jax.config.update("jax_persistent_cache_min_entry_size_bytes", -1)
    except Exception:
        pass
    in_names, out_names, out_avals, zero_outs = [], [], [], []
    for alloc in nc.m.functions[0].allocations:
        if not isinstance(alloc, mybir.MemoryLocationSet):
            continue
        name = alloc.memorylocations[0].name
        if alloc.kind == "ExternalInput":
            if nc.partition_id_tensor is None or \
                    name != nc.partition_id_tensor.name:
                in_names.append(name)
        elif alloc.kind == "ExternalOutput":
            out_names.append(name)
            shape = tuple(alloc.tensor_shape)
            dtype = mybir.dt.np(alloc.dtype)
            out_avals.append(jax.core.ShapedArray(shape, dtype))
            zero_outs.append(np.zeros(shape, dtype))
    all_names = in_names + out_names
    if nc.partition_id_tensor is not None:
        all_names.append(nc.partition_id_tensor.name)

    def _body(*args):
        operands = list(args)
        if nc.partition_id_tensor is not None:
            operands.append(bass2jax.partition_id_tensor())
        return tuple(_bass_exec_p.bind(
            *operands, out_avals=tuple(out_avals), in_names=tuple(all_names),
            out_names=tuple(out_names), lowering_input_output_aliases=(),
            sim_require_finite=True, sim_require_nnan=True, nc=nc))

    devices = jax.devices()[:NCORES]
    mesh = Mesh(np.asarray(devices), ("core",))
    n_io = len(in_names) + len(out_names)
    sharded = jax.jit(
        shard_map(_body, mesh=mesh,
                  in_specs=(PartitionSpec("core"),) * n_io,
                  out_specs=(PartitionSpec("core"),) * len(out_names),
                  check_rep=False),
        keep_unused=True)
    _COMPILED["runner"] = (sharded, in_names, out_names, zero_outs)
    return _COMPILED["runner"]


def _hilo_packed(x):
    """[256, F] fp32 -> (hi, lo) fp8e4m3 pair in [128, 2, F] DoubleRow slot
    layout (dim 1 = the two 128-row C-halves)."""
    hi = x.astype(F8)
    lo = (x - hi.astype(np.float32)).astype(F8)
    F = x.shape[1]

    def pack(x8):
        return np.ascontiguousarray(
            x8.reshape(2, 128, F).transpose(1, 0, 2))
    return pack(hi), pack(lo)


def kernel(feat_A, feat_B, Wq, bq, Wk, bk, Wv, bv, Wo, bo, **_unused):
    f32 = np.float32
    fa = np.asarray(feat_A, f32).reshape(B, C, HW)
    fb = np.asarray(feat_B, f32).reshape(B, C, HW)
    # fold Wk into the Q projection and Wo into the V projection (see
    # module docstring); the (Q-bias . bk) cross term is a per-query
    # constant, which softmax ignores, so it is dropped exactly.
    Wq64 = np.asarray(Wq, np.float64) * SCALE
    Wk64 = np.asarray(Wk, np.float64)
    Wv64 = np.asarray(Wv, np.float64)
    Wo64 = np.asarray(Wo, np.float64)
    wq_t = (Wq64.T @ Wk64).astype(f32)
    wv_t = (Wo64 @ Wv64).T.astype(f32)
    bq_s = ((np.asarray(bq, np.float64) * SCALE) @ Wk64).astype(f32)
    bv_r = (Wo64 @ np.asarray(bv, np.float64)).astype(f32)
    bo_c = np.asarray(bo, f32).reshape(C)

    per_batch = {}
    for b_i in range(B):
        fbh, fbl = _hilo_packed(fb[b_i])
        # V = featB^T @ wv_t + bv_r with ones denominator columns, packed
        # [128, KC, C+2] (partition-major key chunks)
        v = fb[b_i].T @ wv_t + bv_r[None, :]
        vp = np.empty((128, KC, C + 2), BF16)
        vp[:, :, 0:C] = v.reshape(KC, 128, C).transpose(1, 0, 2).astype(BF16)
        vp[:, :, C:C + 2] = np.ones((), BF16)
        per_batch[b_i] = (fbh, fbl, vp)

    in_maps = []
    for c in range(NCORES):
        b_i, qh = c // 2, c % 2
        fbh, fbl, vp = per_batch[b_i]
        a_half = fa[b_i][:, qh * NQ:(qh + 1) * NQ]
        qm = a_half.T @ wq_t + bq_s[None, :]       # [NQ, C]
        qm8h, qm8l = _hilo_packed(np.ascontiguousarray(qm.T))
        in_maps.append({
            "qm8h": qm8h, "qm8l": qm8l,
            "fb8h": fbh, "fb8l": fbl, "vbf": vp,
        })

    try:
        sharded, in_names, out_names, zero_outs = _get_runner()
        concat_in = [np.concatenate([in_maps[c][nm] for c in range(NCORES)],
                                    axis=0) for nm in in_names]
        concat_zeros = [np.zeros((NCORES * z.shape[0], *z.shape[1:]), z.dtype)
                        for z in zero_outs]
        out_arrs = sharded(*concat_in, *concat_zeros)
        res_out = np.asarray(out_arrs[out_names.index("out")]) \
            .reshape(NCORES, 128, QG, 4, C)
    except Exception:
        from concourse.bass_utils import run_bass_kernel_spmd
        res = run_bass_kernel_spmd(_get_nc(), in_maps, list(range(NCORES)))
        res_out = np.stack([np.asarray(res.results[c]["out"])
                            for c in range(NCORES)])
    outf = np.empty((B, C, HW), f32)
    for c in range(NCORES):
        b_i, qh = c // 2, c % 2
        # [128p, QG, 4qs, C] -> [NQ, C] with rows ordered (g, qs, p)
        o = res_out[c].transpose(1, 2, 0, 3).reshape(NQ, C) + bo_c[None, :]
        outf[b_i][:, qh * NQ:(qh + 1) * NQ] = o.T
    return outf.reshape(B, C, 64, 64)


if __name__ == "__main__":
    rng = np.random.default_rng(0)
    ins = {
        "feat_A": rng.standard_normal((B, C, 64, 64), dtype=np.float32),
        "feat_B": rng.standard_normal((B, C, 64, 64), dtype=np.float32),
    }
    for nm in ("q", "k", "v", "o"):
        ins[f"W{nm}"] = rng.standard_normal((C, C), dtype=np.float32) / 16.0
        ins[f"b{nm}"] = np.zeros(C, np.float32)
    o = kernel(**ins)
    print("kernel ran, out shape", o.shape, "mean", float(np.abs(o).mean()))
